# revision 1
# baseline (speedup 1.0000x reference)
"""nn_Cvx_ShortestPathNet — TRN2 Bass kernel, 8-core pure data parallelism.

Math (derived from the reference's Dykstra iteration):
    G = A' pinv(AA') A   (orthogonal projector, 760x760), c = b' pinv(AA') A
    w = MLP(d); t_1 = -w
    for k = 1..K:  corr_k = t_k @ G - c ;  t_{k+1} = max(-w, corr_k)
    y = max(-w - corr_K, 0)
(The invariant t2 + p == -w collapses Dykstra's three-sequence state to a
single iterate.)

On-chip layout is transposed ([n2, B_local], n2 padded 760->768 = 6x128
partition tiles) so the per-iteration matmul runs with M=128 on the PE:
    corr^T[j] = sum_k (G block k,j as lhsT) @ t^T[k]
Per iteration: 36 matmuls (PE) + 6 fused scalar_tensor_tensor ops (DVE):
    t_next[j] = max(psum[j] - c[j], negw[j])

Batch 256 is sharded 32 rows per core; A, G, c and MLP weights replicated.
"""

import json
import numpy as np

import concourse.bass as bass
import concourse.mybir as mybir
import concourse.tile as tile
from concourse.bass_utils import run_bass_kernel_spmd

F32 = mybir.dt.float32
AT = mybir.AluOpType
AF = mybir.ActivationFunctionType

JT = 6          # 768/128 edge-dim tiles
BL = 32         # batch rows per core
HT = 5          # 640/128 hidden tiles
K_ITERS = 100
N_CORES = 8
N2 = 760
MM_DTYPE = mybir.dt.float16   # G and t in fp16: 7.4x faster than fp32 on PE
MM_NP = np.float16

# ---------------------------------------------------------------------------
# This container's walrus build rejects instructions carrying more than one
# sync-wait. Split any multi-wait instruction at the BIR-JSON level: insert
# same-engine NoOps before it, each carrying one of the extra waits (waits
# are sem-ge, so order is irrelevant).
_orig_to_json_bytes = bass.Bass.to_json_bytes
_ctr = [0]


def _split_waits_json(raw: bytes) -> bytes:
    j = json.loads(raw)
    changed = False
    for fn in j.get("functions", []):
        for bb in fn.get("blocks", []):
            out = []
            for inst in bb.get("instructions", []):
                si = inst.get("sync_info") or {}
                waits = si.get("on_wait") or []
                if len(waits) > 1:
                    changed = True
                    for w in waits[:-1]:
                        _ctr[0] += 1
                        out.append({
                            "debug": inst.get("debug", 0),
                            "engine": inst["engine"],
                            "ins": [], "outs": [],
                            "name": f"I-waitsplit-{_ctr[0]}",
                            "opcode": "NoOp",
                            "sync_info": {"on_wait": [w], "on_update": []},
                        })
                    si["on_wait"] = waits[-1:]
                out.append(inst)
            bb["instructions"] = out
    return json.dumps(j).encode() if changed else raw


def _patched_to_json_bytes(self, *a, **k):
    return _split_waits_json(_orig_to_json_bytes(self, *a, **k))


bass.Bass.to_json_bytes = _patched_to_json_bytes


def _build(mm_dtype=F32, k_iters=K_ITERS):
    nc = bass.Bass("TRN2", target_bir_lowering=False, debug=False,
                   num_devices=N_CORES)
    DT = mm_dtype

    g_mat = nc.dram_tensor("g_mat", [128, JT * JT * 128], DT, kind="ExternalInput").ap()
    w2t = nc.dram_tensor("w2t", [128, HT * JT * 128], F32, kind="ExternalInput").ap()
    w1 = nc.dram_tensor("w1", [64, HT * 128], F32, kind="ExternalInput").ap()
    dt_in = nc.dram_tensor("dt_in", [64, BL], F32, kind="ExternalInput").ap()
    ccols = nc.dram_tensor("ccols", [128, JT], F32, kind="ExternalInput").ap()
    b1c = nc.dram_tensor("b1c", [128, HT], F32, kind="ExternalInput").ap()
    nb2c = nc.dram_tensor("nb2c", [128, JT], F32, kind="ExternalInput").ap()
    y_out = nc.dram_tensor("y_out", [128, JT * BL], F32, kind="ExternalOutput").ap()

    with tile.TileContext(nc) as tc:
        with (
            tc.tile_pool(name="const", bufs=1) as cpool,
            tc.tile_pool(name="state", bufs=2) as spool,
            tc.tile_pool(name="psum", bufs=2, space="PSUM") as ppool,
        ):
            dT_sb = cpool.tile([64, BL], F32)
            nc.sync.dma_start(out=dT_sb[:], in_=dt_in[:])
            w1_sb = cpool.tile([64, HT * 128], F32)
            nc.sync.dma_start(out=w1_sb[:], in_=w1[:])
            ccols_sb = cpool.tile([128, JT], F32)
            nc.sync.dma_start(out=ccols_sb[:], in_=ccols[:])
            b1c_sb = cpool.tile([128, HT], F32)
            nc.sync.dma_start(out=b1c_sb[:], in_=b1c[:])
            nb2c_sb = cpool.tile([128, JT], F32)
            nc.sync.dma_start(out=nb2c_sb[:], in_=nb2c[:])
            w2_sb = cpool.tile([128, HT * JT * 128], F32)
            nc.sync.dma_start(out=w2_sb[:], in_=w2t[:])
            # G on the SWDGE path so it overlaps the W2 load
            G_sb = cpool.tile([128, JT * JT * 128], DT)
            nc.gpsimd.dma_start(out=G_sb[:], in_=g_mat[:])

            # MLP: h = leaky_relu(d@W1 + b1), negw = -(h@W2 + b2)
            h_sb = cpool.tile([128, HT * BL], F32)
            for m in range(HT):
                ph = ppool.tile([128, BL], F32, tag="mlp")
                nc.tensor.matmul(out=ph[:], lhsT=w1_sb[:, m * 128:(m + 1) * 128],
                                 rhs=dT_sb[:], start=True, stop=True)
                pre = spool.tile([128, BL], F32, tag="pre", name=f"pre{m}")
                nc.scalar.activation(out=pre[:], in_=ph[:], func=AF.Identity,
                                     bias=b1c_sb[:, m:m + 1], scale=1.0)
                # leaky relu: Lrelu's alpha operand is ignored by this
                # compiler build (hardcodes 0.01), so do max(x, 0.1x) on DVE
                nc.vector.scalar_tensor_tensor(
                    out=h_sb[:, m * BL:(m + 1) * BL], in0=pre[:],
                    scalar=0.1, in1=pre[:], op0=AT.mult, op1=AT.max)
            negw = cpool.tile([128, JT * BL], F32)
            for j in range(JT):
                pw = ppool.tile([128, BL], F32, tag="mlp")
                for k2 in range(HT):
                    nc.tensor.matmul(
                        out=pw[:],
                        lhsT=w2_sb[:, (k2 * JT + j) * 128:(k2 * JT + j + 1) * 128],
                        rhs=h_sb[:, k2 * BL:(k2 + 1) * BL],
                        start=(k2 == 0), stop=(k2 == HT - 1))
                nc.scalar.activation(out=negw[:, j * BL:(j + 1) * BL], in_=pw[:],
                                     func=AF.Identity, bias=nb2c_sb[:, j:j + 1],
                                     scale=-1.0)

            t_cur = [spool.tile([128, BL], DT, tag=f"t{k}", name=f"tcur{k}")
                     for k in range(JT)]
            for k in range(JT):
                nc.vector.tensor_copy(out=t_cur[k][:],
                                      in_=negw[:, k * BL:(k + 1) * BL])

            for it in range(k_iters):
                ps = [ppool.tile([128, 2 * BL], F32, tag=f"ps{p}",
                                 name=f"ps{it}_{p}") for p in range(3)]
                for j in range(JT):
                    pj = ps[j // 2]
                    sl = slice((j % 2) * BL, (j % 2 + 1) * BL)
                    for k in range(JT):
                        nc.tensor.matmul(
                            out=pj[:, sl],
                            lhsT=G_sb[:, (k * JT + j) * 128:(k * JT + j + 1) * 128],
                            rhs=t_cur[k][:],
                            start=(k == 0), stop=(k == JT - 1))
                if it < k_iters - 1:
                    t_nxt = [spool.tile([128, BL], DT, tag=f"t{k}",
                                        name=f"t{it}_{k}") for k in range(JT)]
                    for j in range(JT):
                        nc.vector.scalar_tensor_tensor(
                            out=t_nxt[j][:],
                            in0=ps[j // 2][:, (j % 2) * BL:(j % 2 + 1) * BL],
                            scalar=ccols_sb[:, j:j + 1],
                            in1=negw[:, j * BL:(j + 1) * BL],
                            op0=AT.subtract, op1=AT.max)
                    t_cur = t_nxt
                else:
                    y_sb = cpool.tile([128, JT * BL], F32)
                    for j in range(JT):
                        z = spool.tile([128, BL], F32, tag="z", name=f"z{j}")
                        nc.vector.scalar_tensor_tensor(
                            out=z[:],
                            in0=ps[j // 2][:, (j % 2) * BL:(j % 2 + 1) * BL],
                            scalar=ccols_sb[:, j:j + 1],
                            in1=negw[:, j * BL:(j + 1) * BL],
                            op0=AT.subtract, op1=AT.subtract)
                        nc.scalar.activation(out=y_sb[:, j * BL:(j + 1) * BL],
                                             in_=z[:], func=AF.Relu, scale=-1.0)
                    nc.sync.dma_start(out=y_out[:], in_=y_sb[:])
    return nc


def _host_prepare(d, W1, b1, W2, b2, A, b_eq, mm_np_dtype=np.float32):
    A64 = A.astype(np.float64)
    M = np.linalg.pinv(A64 @ A64.T)
    G = A64.T @ M @ A64
    c = (b_eq.astype(np.float64) @ M) @ A64

    n2 = A.shape[1]
    NP = JT * 128
    G_pad = np.zeros((NP, NP), np.float64)
    G_pad[:n2, :n2] = G
    c_pad = np.zeros(NP, np.float64)
    c_pad[:n2] = c

    g_sb = (G_pad.reshape(JT, 128, JT, 128).transpose(1, 0, 2, 3)
            .reshape(128, JT * JT * 128)).astype(mm_np_dtype)
    ccols = c_pad.reshape(JT, 128).T.astype(np.float32).copy()

    HID = W1.shape[1]
    W2_pad = np.zeros((HID, NP), np.float64)
    W2_pad[:, :n2] = W2.astype(np.float64)
    w2_sb = (W2_pad.reshape(HT, 128, JT, 128).transpose(1, 0, 2, 3)
             .reshape(128, HT * JT * 128)).astype(np.float32)
    b1c = b1.reshape(HT, 128).T.astype(np.float32).copy()
    b2_pad = np.zeros(NP, np.float32)
    b2_pad[:n2] = b2
    nb2c = (-b2_pad).reshape(JT, 128).T.astype(np.float32).copy()

    shared = {"g_mat": g_sb, "w2t": w2_sb, "w1": W1.astype(np.float32),
              "ccols": ccols, "b1c": b1c, "nb2c": nb2c}
    B = d.shape[0]
    bl = B // N_CORES
    in_maps = []
    for i in range(N_CORES):
        dT = d[i * bl:(i + 1) * bl, :].T.astype(np.float32).copy()
        in_maps.append({**shared, "dt_in": dT})
    return in_maps


_nc_cache = {}


def kernel(d, W1, b1, W2, b2, A, b_eq):
    d = np.asarray(d, np.float32)
    W1 = np.asarray(W1, np.float32)
    b1 = np.asarray(b1, np.float32)
    W2 = np.asarray(W2, np.float32)
    b2 = np.asarray(b2, np.float32)
    A = np.asarray(A, np.float32)
    b_eq = np.asarray(b_eq, np.float32)

    if "nc" not in _nc_cache:
        _nc_cache["nc"] = _build(mm_dtype=MM_DTYPE)
    nc = _nc_cache["nc"]

    in_maps = _host_prepare(d, W1, b1, W2, b2, A, b_eq, mm_np_dtype=MM_NP)
    res = run_bass_kernel_spmd(nc, in_maps, list(range(N_CORES)))

    outs = []
    for r in res.results:
        y = (r["y_out"].reshape(128, JT, BL).transpose(2, 1, 0)
             .reshape(BL, JT * 128))
        outs.append(y[:, :N2])
    return np.concatenate(outs, axis=0).astype(np.float32)



# revision 4
# speedup vs baseline: 1.4234x; 1.4234x over previous
"""nn_Cvx_ShortestPathNet — TRN2 Bass kernel, 8-core pure data parallelism.

Math (Dykstra alternating projections, c folded into G via a bias lane):
    G = A' pinv(AA') A  (projector),  c = b' pinv(AA') A
    Ghat[768,768]: Ghat[:760,:760] = G, Ghat[760,:760] = -c, Ghat[760,760] = 1
    negw lane 760 := 1 (via the b2 bias constant), so t lane 760 stays 1 and
    t@Ghat == t@G - c on real lanes.
    t_1 = negw = -MLP(d);  t_{k+1} = max(negw, t_k @ Ghat)   (pure tensor max)
    y = max(negw - t_K @ Ghat, 0) = max(ps, negw) - ps

On-chip layout transposed ([768, B_local], 6x128 partition tiles), B_local =
32 per core. Per iteration: 36 fp16 matmuls + 3 DVE tensor_tensor max ops.

PSUM bank plan: `start=True` clears has_written for the WHOLE bank, so two
accumulation groups may not interleave within a bank. Each pair tile is
[128, 1024] fp32 = TWO banks: group 2p writes cols 480:512 (end of bank 0),
group 2p+1 writes cols 512:544 (start of bank 1). Groups get private banks
(interleave freely) while the pair's DVE max reads one contiguous [128,64].
3 pair tiles (6 banks) + MLP [128,512] x2 (2 banks) = all 8 banks.

The matmul order staggers pair closes (21, 25, 37 of 37 slots) so each DVE
result is ready before the next iteration's matmuls consume it (steady state
R_consume = [2,6,18], R_war = [4,8,20] slot positions; zero PE stalls).

Batch 256 sharded 32 rows per core; Ghat, MLP weights replicated.
"""

import json
import numpy as np

import concourse.bass as bass
import concourse.mybir as mybir
import concourse.tile as tile
from concourse.bass_utils import run_bass_kernel_spmd

F32 = mybir.dt.float32
F16 = mybir.dt.float16
AT = mybir.AluOpType
AF = mybir.ActivationFunctionType

JT = 6          # 768/128 edge-dim tiles
BL = 32         # batch rows per core
HT = 5          # 640/128 hidden tiles
K_ITERS = 100
N_CORES = 8
N2 = 760
NP = JT * 128
PCOL = 480      # pair tile: group 2p at cols 480:512, group 2p+1 at 512:544

# Staggered (j,k) order (sched_gen3, lamc=18/lamw=20): period 37 slots,
# pair closes at 21/25/37.
MM_ORDER = [
    (0, 0), (0, 1), (1, 0), (1, 1), (0, 2), (0, 3), (1, 2), (1, 3), (2, 2),
    (2, 3), (2, 0), (2, 1), (3, 2), (3, 3), (3, 0), (3, 1), (0, 4), (0, 5),
    (1, 4), (1, 5), (2, 4), (2, 5), (3, 4), (3, 5), (4, 4), (4, 5), (4, 2),
    (4, 3), (4, 0), (4, 1), (5, 4), (5, 5), (5, 2), (5, 3), (5, 0), (5, 1),
]
_first = {}
_last = {}
for _pos, (_j, _k) in enumerate(MM_ORDER):
    _first.setdefault(_j, _pos)
    _last[_j] = _pos

# ---------------------------------------------------------------------------
# This container's walrus build rejects instructions carrying more than one
# sync-wait. Split any multi-wait instruction at the BIR-JSON level: insert
# same-engine NoOps before it, each carrying one of the extra waits (waits
# are sem-ge, so order is irrelevant).
_orig_to_json_bytes = bass.Bass.to_json_bytes
_ctr = [0]


def _split_waits_json(raw: bytes) -> bytes:
    j = json.loads(raw)
    changed = False
    for fn in j.get("functions", []):
        for bb in fn.get("blocks", []):
            out = []
            for inst in bb.get("instructions", []):
                si = inst.get("sync_info") or {}
                waits = si.get("on_wait") or []
                if len(waits) > 1:
                    changed = True
                    for w in waits[:-1]:
                        _ctr[0] += 1
                        out.append({
                            "debug": inst.get("debug", 0),
                            "engine": inst["engine"],
                            "ins": [], "outs": [],
                            "name": f"I-waitsplit-{_ctr[0]}",
                            "opcode": "NoOp",
                            "sync_info": {"on_wait": [w], "on_update": []},
                        })
                    si["on_wait"] = waits[-1:]
                out.append(inst)
            bb["instructions"] = out
    return json.dumps(j).encode() if changed else raw


def _patched_to_json_bytes(self, *a, **k):
    return _split_waits_json(_orig_to_json_bytes(self, *a, **k))


bass.Bass.to_json_bytes = _patched_to_json_bytes


def _build(k_iters=K_ITERS):
    nc = bass.Bass("TRN2", target_bir_lowering=False, debug=False,
                   num_devices=N_CORES)

    g_mat = nc.dram_tensor("g_mat", [128, JT * JT * 128], F16, kind="ExternalInput").ap()
    w2t = nc.dram_tensor("w2t", [128, HT * JT * 128], F16, kind="ExternalInput").ap()
    w1 = nc.dram_tensor("w1", [64, HT * 128], F16, kind="ExternalInput").ap()
    dt_in = nc.dram_tensor("dt_in", [64, BL], F16, kind="ExternalInput").ap()
    b1c = nc.dram_tensor("b1c", [128, HT], F32, kind="ExternalInput").ap()
    nb2c = nc.dram_tensor("nb2c", [128, JT], F32, kind="ExternalInput").ap()
    y_out = nc.dram_tensor("y_out", [128, JT * BL], F32, kind="ExternalOutput").ap()

    with tile.TileContext(nc) as tc:
        with (
            tc.tile_pool(name="const", bufs=1) as cpool,
            tc.tile_pool(name="state", bufs=2) as spool,
            tc.tile_pool(name="psum", bufs=1, space="PSUM") as ppool,
            tc.tile_pool(name="psum2", bufs=2, space="PSUM") as p2pool,
        ):
            # --- input DMAs over the 3 DMA queues (sync/scalar HWDGE, SWDGE)
            dT_sb = cpool.tile([64, BL], F16)
            nc.sync.dma_start(out=dT_sb[:], in_=dt_in[:])
            w1_sb = cpool.tile([64, HT * 128], F16)
            nc.sync.dma_start(out=w1_sb[:], in_=w1[:])
            b1c_sb = cpool.tile([128, HT], F32)
            nc.sync.dma_start(out=b1c_sb[:], in_=b1c[:])
            nb2c_sb = cpool.tile([128, JT], F32)
            nc.sync.dma_start(out=nb2c_sb[:], in_=nb2c[:])
            w2_sb = cpool.tile([128, HT * JT * 128], F16)
            wc = HT * JT * 128 // 2
            for i, eng in enumerate([nc.sync, nc.scalar]):
                eng.dma_start(out=w2_sb[:, i * wc:(i + 1) * wc],
                              in_=w2t[:, i * wc:(i + 1) * wc])
            G_sb = cpool.tile([128, JT * JT * 128], F16)
            gq = [nc.gpsimd, nc.gpsimd, nc.gpsimd, nc.gpsimd, nc.sync, nc.scalar]
            for k in range(JT):
                sl = slice(k * JT * 128, (k + 1) * JT * 128)
                gq[k].dma_start(out=G_sb[:, sl], in_=g_mat[:, sl])

            # --- MLP: h = leaky_relu(d@W1 + b1), negw = -(h@W2 + b2) -------
            h16 = cpool.tile([128, HT * BL], F16)
            for m in range(HT):
                ph = p2pool.tile([128, 512], F32, tag="mlp", name=f"ph{m}")
                nc.tensor.matmul(out=ph[:, :BL],
                                 lhsT=w1_sb[:, m * 128:(m + 1) * 128],
                                 rhs=dT_sb[:], start=True, stop=True)
                pre = spool.tile([128, BL], F32, tag="pre", name=f"pre{m}")
                nc.scalar.activation(out=pre[:], in_=ph[:, :BL], func=AF.Identity,
                                     bias=b1c_sb[:, m:m + 1], scale=1.0)
                # leaky relu = max(x, 0.1x) on DVE
                nc.vector.scalar_tensor_tensor(
                    out=h16[:, m * BL:(m + 1) * BL], in0=pre[:],
                    scalar=0.1, in1=pre[:], op0=AT.mult, op1=AT.max)
            negw = cpool.tile([128, JT * BL], F32)
            for j in range(JT):
                pw = p2pool.tile([128, 512], F32, tag="mlp", name=f"pw{j}")
                for k2 in range(HT):
                    nc.tensor.matmul(
                        out=pw[:, :BL],
                        lhsT=w2_sb[:, (k2 * JT + j) * 128:(k2 * JT + j + 1) * 128],
                        rhs=h16[:, k2 * BL:(k2 + 1) * BL],
                        start=(k2 == 0), stop=(k2 == HT - 1))
                # negw = -(ps + b2); lane 760 gets +1 via nb2c[120,5] = 1
                nc.scalar.activation(out=negw[:, j * BL:(j + 1) * BL],
                                     in_=pw[:, :BL], func=AF.Identity,
                                     bias=nb2c_sb[:, j:j + 1], scale=-1.0)

            # pair PSUM tiles: 2 banks each; groups 2p / 2p+1 in separate banks
            ps = [ppool.tile([128, 1024], F32, tag=f"ps{p}", name=f"ps{p}")
                  for p in range(3)]

            def out_ap(j):
                col = PCOL + (j % 2) * BL
                return ps[j // 2][:, col:col + BL]

            # t_1 = negw (fp16); lane 760 == 1 already via negw
            t_cur = [spool.tile([128, 2 * BL], F16, tag=f"t{p}", name=f"t0_{p}")
                     for p in range(3)]
            for p in range(3):
                nc.vector.tensor_copy(out=t_cur[p][:],
                                      in_=negw[:, p * 2 * BL:(p + 1) * 2 * BL])

            # --- Dykstra iterations ---------------------------------------
            for it in range(k_iters):
                last_iter = it == k_iters - 1
                if not last_iter:
                    t_nxt = [spool.tile([128, 2 * BL], F16, tag=f"t{p}",
                                        name=f"t{it + 1}_{p}") for p in range(3)]
                else:
                    tl = cpool.tile([128, JT * BL], F32)
                    y_sb = cpool.tile([128, JT * BL], F32)
                left = {j: 6 for j in range(JT)}
                for pos, (j, k) in enumerate(MM_ORDER):
                    nc.tensor.matmul(
                        out=out_ap(j),
                        lhsT=G_sb[:, (k * JT + j) * 128:(k * JT + j + 1) * 128],
                        rhs=t_cur[k // 2][:, (k % 2) * BL:(k % 2 + 1) * BL],
                        start=(pos == _first[j]), stop=(pos == _last[j]))
                    left[j] -= 1
                    p = j // 2
                    if left[2 * p] == 0 and left[2 * p + 1] == 0:
                        left[2 * p] = left[2 * p + 1] = -1  # fire once
                        pss = ps[p][:, PCOL:PCOL + 2 * BL]
                        nws = negw[:, p * 2 * BL:(p + 1) * 2 * BL]
                        if not last_iter:
                            nc.vector.tensor_tensor(out=t_nxt[p][:], in0=pss,
                                                    in1=nws, op=AT.max)
                        else:
                            sl = slice(p * 2 * BL, (p + 1) * 2 * BL)
                            nc.vector.tensor_tensor(out=tl[:, sl], in0=pss,
                                                    in1=nws, op=AT.max)
                            nc.vector.tensor_tensor(out=y_sb[:, sl],
                                                    in0=tl[:, sl], in1=pss,
                                                    op=AT.subtract)
                if not last_iter:
                    t_cur = t_nxt
                else:
                    nc.sync.dma_start(out=y_out[:], in_=y_sb[:])
    return nc


def _host_prepare(d, W1, b1, W2, b2, A, b_eq):
    A64 = A.astype(np.float64)
    M = np.linalg.pinv(A64 @ A64.T)
    G = A64.T @ M @ A64
    c = (b_eq.astype(np.float64) @ M) @ A64

    n2 = A.shape[1]
    Ghat = np.zeros((NP, NP), np.float64)
    Ghat[:n2, :n2] = G
    Ghat[n2, :n2] = -c          # bias lane row
    Ghat[n2, n2] = 1.0

    g_sb = (Ghat.reshape(JT, 128, JT, 128).transpose(1, 0, 2, 3)
            .reshape(128, JT * JT * 128)).astype(np.float16)

    HID = W1.shape[1]
    W2_pad = np.zeros((HID, NP), np.float64)
    W2_pad[:, :n2] = W2.astype(np.float64)
    w2_sb = (W2_pad.reshape(HT, 128, JT, 128).transpose(1, 0, 2, 3)
             .reshape(128, HT * JT * 128)).astype(np.float16)
    b1c = b1.reshape(HT, 128).T.astype(np.float32).copy()
    b2_pad = np.zeros(NP, np.float32)
    b2_pad[:n2] = b2
    nb2c = (-b2_pad).reshape(JT, 128).T.astype(np.float32).copy()
    nb2c[n2 - 5 * 128, 5] = 1.0   # lane 760 -> partition 120, block 5

    shared = {"g_mat": g_sb, "w2t": w2_sb, "w1": W1.astype(np.float16),
              "b1c": b1c, "nb2c": nb2c}
    B = d.shape[0]
    bl = B // N_CORES
    in_maps = []
    for i in range(N_CORES):
        dT = d[i * bl:(i + 1) * bl, :].T.astype(np.float16).copy()
        in_maps.append({**shared, "dt_in": dT})
    return in_maps


_nc_cache = {}


def kernel(d, W1, b1, W2, b2, A, b_eq):
    d = np.asarray(d, np.float32)
    W1 = np.asarray(W1, np.float32)
    b1 = np.asarray(b1, np.float32)
    W2 = np.asarray(W2, np.float32)
    b2 = np.asarray(b2, np.float32)
    A = np.asarray(A, np.float32)
    b_eq = np.asarray(b_eq, np.float32)

    if "nc" not in _nc_cache:
        _nc_cache["nc"] = _build()
    nc = _nc_cache["nc"]

    in_maps = _host_prepare(d, W1, b1, W2, b2, A, b_eq)
    res = run_bass_kernel_spmd(nc, in_maps, list(range(N_CORES)))

    outs = []
    for r in res.results:
        y = (r["y_out"].reshape(128, JT, BL).transpose(2, 1, 0)
             .reshape(BL, JT * 128))
        outs.append(y[:, :N2])
    return np.concatenate(outs, axis=0).astype(np.float32)


# revision 9
# speedup vs baseline: 1.6593x; 1.1657x over previous
"""nn_Cvx_ShortestPathNet — TRN2 Bass kernel, 8-core pure data parallelism.

Math (Dykstra alternating projections, c folded into G via a bias lane):
    G = A' pinv(AA') A  (projector),  c = b' pinv(AA') A
    Ghat[768,768]: Ghat[:760,:760] = G, Ghat[760,:760] = -c, Ghat[760,760] = 1
    negw lane 760 := 1 (via the b2 bias constant), so t lane 760 stays 1 and
    t@Ghat == t@G - c on real lanes.
    t_1 = negw = -MLP(d);  t_{k+1} = max(negw, t_k @ Ghat)   (pure tensor max)
    y = max(negw - t_K @ Ghat, 0) = max(ps, negw) - ps

On-chip layout transposed ([768, B_local], 6x128 partition tiles), B_local =
32 per core. Per iteration: 36 fp16 matmuls + 3 DVE tensor_tensor max ops.

PSUM bank plan: `start=True` clears has_written for the WHOLE bank, so two
accumulation groups may not interleave within a bank. Each pair tile is
[128, 1024] fp32 = TWO banks: group 2p writes cols 480:512 (end of bank 0),
group 2p+1 writes cols 512:544 (start of bank 1). Groups get private banks
(interleave freely) while the pair's DVE max reads one contiguous [128,64].
3 pair tiles (6 banks) + MLP [128,512] x2 (2 banks) = all 8 banks.

The matmul order staggers pair closes (21, 25, 37 of 37 slots) so each DVE
result is ready before the next iteration's matmuls consume it (steady state
R_consume = [2,6,18], R_war = [4,8,20] slot positions; zero PE stalls).

Batch 256 sharded 32 rows per core; Ghat, MLP weights replicated.
"""

import json
import numpy as np

import concourse.bass as bass
import concourse.mybir as mybir
import concourse.tile as tile
from concourse.bass_utils import run_bass_kernel_spmd

F32 = mybir.dt.float32
F16 = mybir.dt.float16
AT = mybir.AluOpType
AF = mybir.ActivationFunctionType

JT = 6          # 768/128 edge-dim tiles
BL = 32         # batch rows per core
HT = 5          # 640/128 hidden tiles
K_ITERS = 100
N_CORES = 8
N2 = 760
NP = JT * 128
PCOL = 480      # pair tile: group 2p at cols 480:512, group 2p+1 at 512:544

# Staggered (j,k) order (sched_gen3, lamc=18/lamw=20): period 37 slots,
# pair closes at 21/25/37.
MM_ORDER = [
    (0, 0), (0, 1), (1, 0), (1, 1), (0, 2), (0, 3), (1, 2), (1, 3), (2, 2),
    (2, 3), (2, 0), (2, 1), (3, 2), (3, 3), (3, 0), (3, 1), (0, 4), (0, 5),
    (1, 4), (1, 5), (2, 4), (2, 5), (3, 4), (3, 5), (4, 4), (4, 5), (4, 2),
    (4, 3), (4, 0), (4, 1), (5, 4), (5, 5), (5, 2), (5, 3), (5, 0), (5, 1),
]
_first = {}
_last = {}
for _pos, (_j, _k) in enumerate(MM_ORDER):
    _first.setdefault(_j, _pos)
    _last[_j] = _pos

# ---------------------------------------------------------------------------
# This container's walrus build rejects instructions carrying more than one
# sync-wait. Split any multi-wait instruction at the BIR-JSON level: insert
# same-engine NoOps before it, each carrying one of the extra waits (waits
# are sem-ge, so order is irrelevant).
_orig_to_json_bytes = bass.Bass.to_json_bytes
_ctr = [0]


def _split_waits_json(raw: bytes) -> bytes:
    j = json.loads(raw)
    changed = False
    for fn in j.get("functions", []):
        for bb in fn.get("blocks", []):
            out = []
            for inst in bb.get("instructions", []):
                si = inst.get("sync_info") or {}
                waits = si.get("on_wait") or []
                if len(waits) > 1:
                    changed = True
                    for w in waits[:-1]:
                        _ctr[0] += 1
                        out.append({
                            "debug": inst.get("debug", 0),
                            "engine": inst["engine"],
                            "ins": [], "outs": [],
                            "name": f"I-waitsplit-{_ctr[0]}",
                            "opcode": "NoOp",
                            "sync_info": {"on_wait": [w], "on_update": []},
                        })
                    si["on_wait"] = waits[-1:]
                out.append(inst)
            bb["instructions"] = out
    return json.dumps(j).encode() if changed else raw


def _patched_to_json_bytes(self, *a, **k):
    return _split_waits_json(_orig_to_json_bytes(self, *a, **k))


bass.Bass.to_json_bytes = _patched_to_json_bytes


def _build(k_iters=K_ITERS):
    nc = bass.Bass("TRN2", target_bir_lowering=False, debug=False,
                   num_devices=N_CORES)

    g_mat = nc.dram_tensor("g_mat", [128, JT * JT * 128], F16, kind="ExternalInput").ap()
    w2t = nc.dram_tensor("w2t", [128, HT * JT * 128], F16, kind="ExternalInput").ap()
    w1 = nc.dram_tensor("w1", [64, HT * 128], F16, kind="ExternalInput").ap()
    dt_in = nc.dram_tensor("dt_in", [64, BL], F16, kind="ExternalInput").ap()
    b1c = nc.dram_tensor("b1c", [128, HT], F32, kind="ExternalInput").ap()
    nb2c = nc.dram_tensor("nb2c", [128, JT], F32, kind="ExternalInput").ap()
    y_out = nc.dram_tensor("y_out", [128, JT * BL], F16, kind="ExternalOutput").ap()

    with tile.TileContext(nc) as tc:
        with (
            tc.tile_pool(name="const", bufs=1) as cpool,
            tc.tile_pool(name="state", bufs=2) as spool,
            tc.tile_pool(name="psum", bufs=1, space="PSUM") as ppool,
            tc.tile_pool(name="psum2", bufs=2, space="PSUM") as p2pool,
        ):
            # --- input DMAs over the 3 DMA queues (sync/scalar HWDGE, SWDGE)
            dT_sb = cpool.tile([64, BL], F16)
            nc.sync.dma_start(out=dT_sb[:], in_=dt_in[:])
            w1_sb = cpool.tile([64, HT * 128], F16)
            nc.sync.dma_start(out=w1_sb[:], in_=w1[:])
            b1c_sb = cpool.tile([128, HT], F32)
            nc.sync.dma_start(out=b1c_sb[:], in_=b1c[:])
            nb2c_sb = cpool.tile([128, JT], F32)
            nc.sync.dma_start(out=nb2c_sb[:], in_=nb2c[:])
            w2_sb = cpool.tile([128, HT * JT * 128], F16)
            wc = HT * JT * 128 // 2
            for i, eng in enumerate([nc.sync, nc.scalar]):
                eng.dma_start(out=w2_sb[:, i * wc:(i + 1) * wc],
                              in_=w2t[:, i * wc:(i + 1) * wc])
            G_sb = cpool.tile([128, JT * JT * 128], F16)
            gq = [nc.gpsimd, nc.gpsimd, nc.gpsimd, nc.gpsimd, nc.sync, nc.scalar]
            for k in range(JT):
                sl = slice(k * JT * 128, (k + 1) * JT * 128)
                gq[k].dma_start(out=G_sb[:, sl], in_=g_mat[:, sl])

            # --- HAM warm-up: dummy matmuls keep the PE busy through the
            # DMA phase so the clock gate reaches K=8/8 before the real work
            warm_ps = p2pool.tile([128, 512], F32, tag="mlp", name="warm")
            for _ in range(140):
                nc.tensor.matmul(out=warm_ps[:32, :BL], lhsT=dT_sb[:, :BL],
                                 rhs=dT_sb[:], start=True, stop=True)

            # --- MLP: h = leaky_relu(d@W1 + b1), negw = -(h@W2 + b2) -------
            h16 = cpool.tile([128, HT * BL], F16)
            for m in range(HT):
                ph = p2pool.tile([128, 512], F32, tag="mlp", name=f"ph{m}")
                nc.tensor.matmul(out=ph[:, :BL],
                                 lhsT=w1_sb[:, m * 128:(m + 1) * 128],
                                 rhs=dT_sb[:], start=True, stop=True)
                pre = spool.tile([128, BL], F32, tag="pre", name=f"pre{m}")
                nc.scalar.activation(out=pre[:], in_=ph[:, :BL], func=AF.Identity,
                                     bias=b1c_sb[:, m:m + 1], scale=1.0)
                # leaky relu = max(x, 0.1x) on DVE
                nc.vector.scalar_tensor_tensor(
                    out=h16[:, m * BL:(m + 1) * BL], in0=pre[:],
                    scalar=0.1, in1=pre[:], op0=AT.mult, op1=AT.max)
            negw = cpool.tile([128, JT * BL], F32)
            for j in range(JT):
                pw = p2pool.tile([128, 512], F32, tag="mlp", name=f"pw{j}")
                for k2 in range(HT):
                    nc.tensor.matmul(
                        out=pw[:, :BL],
                        lhsT=w2_sb[:, (k2 * JT + j) * 128:(k2 * JT + j + 1) * 128],
                        rhs=h16[:, k2 * BL:(k2 + 1) * BL],
                        start=(k2 == 0), stop=(k2 == HT - 1))
                # negw = -(ps + b2); lane 760 gets +1 via nb2c[120,5] = 1
                nc.scalar.activation(out=negw[:, j * BL:(j + 1) * BL],
                                     in_=pw[:, :BL], func=AF.Identity,
                                     bias=nb2c_sb[:, j:j + 1], scale=-1.0)

            # fp16 negw copy: loop DVE max ops run all-16-bit on the SBUF side
            negw16 = cpool.tile([128, JT * BL], F16)
            nc.vector.tensor_copy(out=negw16[:], in_=negw[:])

            # pair PSUM tiles: 2 banks each; groups 2p / 2p+1 in separate banks
            ps = [ppool.tile([128, 1024], F32, tag=f"ps{p}", name=f"ps{p}")
                  for p in range(3)]

            def out_ap(j):
                col = PCOL + (j % 2) * BL
                return ps[j // 2][:, col:col + BL]

            # t_1 = negw (fp16); lane 760 == 1 already via negw
            t_cur = [spool.tile([128, 2 * BL], F16, tag=f"t{p}", name=f"t0_{p}")
                     for p in range(3)]
            for p in range(3):
                nc.vector.tensor_copy(out=t_cur[p][:],
                                      in_=negw16[:, p * 2 * BL:(p + 1) * 2 * BL])

            # --- Dykstra iterations ---------------------------------------
            for it in range(k_iters):
                last_iter = it == k_iters - 1
                if not last_iter:
                    t_nxt = [spool.tile([128, 2 * BL], F16, tag=f"t{p}",
                                        name=f"t{it + 1}_{p}") for p in range(3)]
                else:
                    tl = cpool.tile([128, JT * BL], F32)
                    y_sb = cpool.tile([128, JT * BL], F16)
                left = {j: 6 for j in range(JT)}
                for pos, (j, k) in enumerate(MM_ORDER):
                    nc.tensor.matmul(
                        out=out_ap(j),
                        lhsT=G_sb[:, (k * JT + j) * 128:(k * JT + j + 1) * 128],
                        rhs=t_cur[k // 2][:, (k % 2) * BL:(k % 2 + 1) * BL],
                        start=(pos == _first[j]), stop=(pos == _last[j]))
                    left[j] -= 1
                    p = j // 2
                    if left[2 * p] == 0 and left[2 * p + 1] == 0:
                        left[2 * p] = left[2 * p + 1] = -1  # fire once
                        pss = ps[p][:, PCOL:PCOL + 2 * BL]
                        sl = slice(p * 2 * BL, (p + 1) * 2 * BL)
                        if not last_iter:
                            nc.vector.tensor_tensor(out=t_nxt[p][:], in0=pss,
                                                    in1=negw16[:, sl],
                                                    op=AT.max)
                        else:
                            nc.vector.tensor_tensor(out=tl[:, sl], in0=pss,
                                                    in1=negw[:, sl], op=AT.max)
                            nc.vector.tensor_tensor(out=y_sb[:, sl],
                                                    in0=tl[:, sl], in1=pss,
                                                    op=AT.subtract)
                if not last_iter:
                    t_cur = t_nxt
                else:
                    nc.sync.dma_start(out=y_out[:], in_=y_sb[:])
    return nc


def _host_prepare(d, W1, b1, W2, b2, A, b_eq):
    A64 = A.astype(np.float64)
    M = np.linalg.pinv(A64 @ A64.T)
    G = A64.T @ M @ A64
    c = (b_eq.astype(np.float64) @ M) @ A64

    n2 = A.shape[1]
    Ghat = np.zeros((NP, NP), np.float64)
    Ghat[:n2, :n2] = G
    Ghat[n2, :n2] = -c          # bias lane row
    Ghat[n2, n2] = 1.0

    g_sb = (Ghat.reshape(JT, 128, JT, 128).transpose(1, 0, 2, 3)
            .reshape(128, JT * JT * 128)).astype(np.float16)

    HID = W1.shape[1]
    W2_pad = np.zeros((HID, NP), np.float64)
    W2_pad[:, :n2] = W2.astype(np.float64)
    w2_sb = (W2_pad.reshape(HT, 128, JT, 128).transpose(1, 0, 2, 3)
             .reshape(128, HT * JT * 128)).astype(np.float16)
    b1c = b1.reshape(HT, 128).T.astype(np.float32).copy()
    b2_pad = np.zeros(NP, np.float32)
    b2_pad[:n2] = b2
    nb2c = (-b2_pad).reshape(JT, 128).T.astype(np.float32).copy()
    nb2c[n2 - 5 * 128, 5] = 1.0   # lane 760 -> partition 120, block 5

    shared = {"g_mat": g_sb, "w2t": w2_sb, "w1": W1.astype(np.float16),
              "b1c": b1c, "nb2c": nb2c}
    B = d.shape[0]
    bl = B // N_CORES
    in_maps = []
    for i in range(N_CORES):
        dT = d[i * bl:(i + 1) * bl, :].T.astype(np.float16).copy()
        in_maps.append({**shared, "dt_in": dT})
    return in_maps


_nc_cache = {}


def kernel(d, W1, b1, W2, b2, A, b_eq):
    d = np.asarray(d, np.float32)
    W1 = np.asarray(W1, np.float32)
    b1 = np.asarray(b1, np.float32)
    W2 = np.asarray(W2, np.float32)
    b2 = np.asarray(b2, np.float32)
    A = np.asarray(A, np.float32)
    b_eq = np.asarray(b_eq, np.float32)

    if "nc" not in _nc_cache:
        _nc_cache["nc"] = _build()
    nc = _nc_cache["nc"]

    in_maps = _host_prepare(d, W1, b1, W2, b2, A, b_eq)
    res = run_bass_kernel_spmd(nc, in_maps, list(range(N_CORES)))

    outs = []
    for r in res.results:
        y = (r["y_out"].reshape(128, JT, BL).transpose(2, 1, 0)
             .reshape(BL, JT * 128))
        outs.append(y[:, :N2])
    return np.concatenate(outs, axis=0).astype(np.float32)
